# revision 58
# baseline (speedup 1.0000x reference)
"""GraphSAGE GNN (N=100k, E=600k, D=128, L=4) on 8 Trainium2 NeuronCores.

Strategy (memory-regime):
- Layer 3 of the reference is dead code (its output never reaches node_rep):
  only 3 SAGE layers are computed.
- Nodes sharded contiguously: core c owns dsts [c*12500,(c+1)*12500), padded
  to 12544 rows. Edges sorted by dst on host (graph partitioning): each
  64-dst group gets 3 regular 128-edge tiles; overflow edges go to S shared
  per-chunk spill tiles (one-hot spans the whole 512-dst chunk) -> ~92%
  slot fill with an identical SPMD instruction stream on all cores.
- Mean-aggregation = PE matmuls: psum[feat,dst] += G_tile.T @ IND_tile where
  G = gathered source rows (fp8e4m3, 128B/row) and IND holds exact 0/1
  one-hot columns in fp8; deg_inv applied afterwards as an elementwise
  multiply against a bf16 tile broadcast on-device from a [1, PADN] row
  via a K=1 PE outer product (exact bf16 mean, fp8 only on transported h).
  Spill-tile one-hots are built on-device (gpsimd iota==col compare) from
  a tiny column-index array instead of streaming 3.2MB of mostly zeros.
- Indirect gathers are batched one chunk (26 tiles, 3328 rows) per SWDGE
  instruction (64KB descriptor scratch) to amortize the ~1us fixed
  desc-gen cost on Pool; transfers run on the 16 DMA engines.
- Layer 0's gather is precomputed on host (x[src] in fp8) and streamed in
  partition-major layout [128, T*128] so DMA runs are 4KB (full bandwidth).
- h tables replicated via per-slab AllGather (fp8, slab-major table layout
  so each slab's output is contiguous), overlapped with remaining chunks.
- Node-major shard rows computed directly by transposed matmuls (lhsT =
  agg/HT 128-node blocks, rhs = weights, bias via a K=1 ones-matmul into
  PSUM, bias-less DVE relu) — no PE transposes, and shard writes never
  wait on the Act engine's chain.
- BN (eval) folded into weights/bias on host. Activations kept transposed
  [feat, node]; per-layer JK pooling via the ACT accum_out side-output.
- Final global pool partials [128] per core summed on host + tiny MLP head
  on host (0.1% of FLOPs).
"""
import numpy as np
import ml_dtypes
from contextlib import ExitStack

import concourse.bass as bass
import concourse.tile as tile
import concourse.tile as tile_mod
from concourse import mybir
from concourse.vector_clock import ScopedClock

# ---------------------------------------------------------------------------
# Walrus workaround: this compiler rejects >1 sem wait on CTRL_NO instructions
# (Drain/NoOp). Tile's final drain carries one wait per active proc — split
# them one-per-nop and emit a waitless drain.
def _drain_and_barrier(self, tick_clock, wait_clock):
    probe = self.nc.sync.nop(hint="pre_drain_waits", nofuse=True)
    wait_clock.add_sem_waits(probe.ins, ScopedClock({None: tick_clock.global_clock}))
    si = probe.ins.sync_info
    waits = list(si.on_wait) if si is not None else []
    if len(waits) > 1:
        probe.ins.sync_info = mybir.SyncInfo(on_wait=waits[:1], on_update=[])
        for w in waits[1:]:
            extra = self.nc.sync.nop(hint="pre_drain_waits_x", nofuse=True)
            extra.ins.sync_info = mybir.SyncInfo(on_wait=[w], on_update=[])
    self.nc.sync.drain()
    self.nc.all_engine_barrier()
    assert self.sems is not None
    popped = self.nc._tile_sem_poison_stack.pop()
    assert popped is self._sem_poison
    self.nc.clear_and_free_semaphores(list(self.sems.allocated().values()))
    self.nc.all_engine_barrier()


tile_mod.TileContext._drain_and_barrier = _drain_and_barrier


def _split_multi_waits(nc):
    """This walrus build allows at most ONE sem wait per instruction. Tile
    emits several on some. Split: carrier nops (same engine, program order
    preserved) take all but the last wait."""
    targets = []
    for f in nc.m.functions:
        for blk in f.blocks:
            for inst in blk.instructions:
                si = inst.sync_info
                if si is not None and len(si.on_wait) > 1:
                    targets.append((blk, inst))
    if not targets:
        return
    carriers = {}  # inst name -> list of carrier insts
    created = []
    for blk, inst in targets:
        waits = list(inst.sync_info.on_wait)
        cs = []
        for w in waits[:-1]:
            c = nc.engines[inst.engine].nop(hint="wsplit", nofuse=True)
            c.ins.sync_info = mybir.SyncInfo(on_wait=[w], on_update=[])
            cs.append(c.ins)
            created.append(c.ins.name)
        inst.sync_info = mybir.SyncInfo(
            on_wait=[waits[-1]], on_update=list(inst.sync_info.on_update))
        carriers[inst.name] = cs
    created = set(created)
    for f in nc.m.functions:
        for blk in f.blocks:
            insts = list(blk.instructions)
            new = []
            changed = False
            for inst in insts:
                if inst.name in created:
                    changed = True
                    continue  # remove from tail position
                if inst.name in carriers:
                    new.extend(carriers[inst.name])
                    changed = True
                new.append(inst)
            if changed:
                blk.instructions = new

# ---------------------------------------------------------------------------
N, E, D, L = 100000, 600000, 128, 4
P = 8
NPC = N // P            # 12500 nodes per core
PADN = 12544            # padded to 98*128
V = PADN * P            # padded global table rows
GW = 64                 # dst-group width
NGRP = PADN // GW       # 196 groups per core
BN_EPS = 1e-5
NCHUNK = 25             # 24x512 + 1x256 node chunks
K_R = 3                 # regular 128-edge tiles per 64-dst group
SLAB_CHUNKS = [8, 8, 4, 2, 2, 1]  # chunks per AllGather slab (tapered tail)
SLAB_FIRST = [0, 8, 16, 20, 22, 24]
SLAB_NS0 = [0, 4096, 8192, 10240, 11264, 12288]
SLAB_NSW = [4096, 4096, 2048, 1024, 1024, 256]
bf16 = mybir.dt.bfloat16
f8 = mybir.dt.float8e4
f32 = mybir.dt.float32
i32 = mybir.dt.int32
nbf = ml_dtypes.bfloat16
nf8 = ml_dtypes.float8_e4m3


def _host_prep(x, edge_index):
    src = np.asarray(edge_index[0], dtype=np.int64)
    dst = np.asarray(edge_index[1], dtype=np.int64)
    deg = np.bincount(dst, minlength=N).astype(np.float64)
    deg_inv = np.where(deg > 0, 1.0 / np.maximum(deg, 1.0), 0.0).astype(np.float32)

    order = np.argsort(dst, kind="stable")
    ds = dst[order]
    ss = src[order]
    core_bounds = np.searchsorted(ds, np.arange(P + 1) * NPC)

    # per (core, group) edge counts. Each 64-dst group gets K_R=3 regular
    # tiles (384 slots); overflow edges spill into S shared per-chunk tiles
    # whose one-hot IND spans the whole 512-dst chunk. S is data-derived.
    percore = []
    max_spill = 0
    for c in range(P):
        lo, hi = core_bounds[c], core_bounds[c + 1]
        l = (ds[lo:hi] - c * NPC).astype(np.int64)
        s = ss[lo:hi]
        gb = np.searchsorted(l, np.arange(NGRP + 1) * GW)
        cnt = np.diff(gb)
        ov = np.maximum(cnt - K_R * 128, 0)
        for ch in range(NCHUNK):
            g0, g1 = ch * 8, min(ch * 8 + 8, NGRP)
            max_spill = max(max_spill, int(ov[g0:g1].sum()))
        percore.append((l, s, gb))
    S = (max_spill + 127) // 128
    TSTR = 8 * K_R + S           # tile stride per chunk
    T = 24 * TSTR + 4 * K_R + S  # 24 full chunks + final half chunk
    TREG = NGRP * K_R            # regular tiles total (588)

    # table rows are slab-major (slab, core, node-in-slab) so each slab's
    # AllGather lands in one contiguous output range
    _b = np.asarray(SLAB_NS0 + [PADN])
    _w = np.asarray(SLAB_NSW)
    def _srow(glob):
        sc = glob // NPC
        local = glob % NPC
        si = np.searchsorted(_b, local, "right") - 1
        return 8 * _b[si] + sc * _w[si] + (local - _b[si])

    xf = np.asarray(x, dtype=np.float32)

    ins = []
    for c in range(P):
        lo, hi = core_bounds[c], core_bounds[c + 1]
        l, s, gb = percore[c]
        sr = _srow(ss[lo:hi])

        idx_t = np.zeros((T, 128), np.int32)            # tile, edge-in-tile
        ind_t = np.zeros((TREG, 128, GW), np.float32)   # regular tiles
        spcol = np.full((NCHUNK * S, 128), 1e9, np.float32)  # spill dst cols
        xsrc_t = np.zeros((T * 128,), np.int64) - 1     # original src per slot
        for ch in range(NCHUNK):
            ngr = 8 if ch < 24 else 4
            nreg = ngr * K_R
            t0 = ch * TSTR
            spfill = 0
            for gi in range(ngr):
                g = ch * 8 + gi
                e0, e1 = gb[g], gb[g + 1]
                n = e1 - e0
                if n == 0:
                    continue
                rows = sr[e0:e1]
                gcols = (l[e0:e1] - g * GW).astype(np.int64)
                ccols = (l[e0:e1] - ch * 512).astype(np.int64)
                nr = min(n, K_R * 128)
                for kk in range((nr + 127) // 128):
                    a, b = kk * 128, min((kk + 1) * 128, nr)
                    pp = np.arange(a, b) - a
                    t = t0 + gi * K_R + kk
                    rt = (ch * 24 if ch < 24 else 576) + gi * K_R + kk
                    idx_t[t, pp] = rows[a:b]
                    ind_t[rt, pp, gcols[a:b]] = 1.0
                    xsrc_t[t * 128 + pp] = s[e0 + a:e0 + b]
                for e in range(nr, n):      # spill
                    k, slot = spfill // 128, spfill % 128
                    t = t0 + nreg + k
                    idx_t[t, slot] = rows[e]
                    spcol[ch * S + k, slot] = ccols[e]
                    xsrc_t[t * 128 + slot] = s[e0 + e]
                    spfill += 1
            assert spfill <= S * 128

        idx_in = np.ascontiguousarray(idx_t.T)                    # [128, T]
        ind_in = np.ascontiguousarray(
            ind_t.transpose(1, 0, 2).reshape(128, TREG * GW)).astype(nf8)
        scol_in = np.ascontiguousarray(spcol.T)                   # [128, 25*S]
        dvrow = np.zeros((PADN,), np.float32)
        dvrow[:NPC] = deg_inv[c * NPC:(c + 1) * NPC]
        dv_in = dvrow[None, :].astype(nbf)
        xg0 = np.zeros((T * 128, D), nf8)
        valid = xsrc_t >= 0
        xg0[valid] = xf[xsrc_t[valid]].astype(nf8)
        # partition-major: xg[p, t*128+f] = x[src(tile t, slot p)][f]
        xg = np.ascontiguousarray(
            xg0.reshape(T, 128, D).transpose(1, 0, 2).reshape(128, T * D))

        xT = np.zeros((128, PADN), np.float32)
        xT[:, :NPC] = xf[c * NPC:(c + 1) * NPC].T
        ins.append({
            "idx": idx_in,
            "ind": ind_in,
            "scol": scol_in,
            "iota": np.ascontiguousarray(np.broadcast_to(
                np.arange(512, dtype=np.float32)[None, :], (128, 512))),
            "xg": xg,
            "xT": np.ascontiguousarray(xT.astype(nbf)),
            "dvr": np.ascontiguousarray(dv_in[:1]),
        })
    return ins, S, deg_inv


def _fold_weights(lin_l_w, lin_l_b, lin_r_w, bn_w, bn_b):
    inv_std = 1.0 / np.sqrt(1.0 + BN_EPS)
    wl, wr, bb = [], [], []
    for l in range(3):
        scale = (np.asarray(bn_w[l], np.float32) * inv_std)
        wl.append((np.asarray(lin_l_w[l], np.float32) * scale[:, None]).T)
        wr.append((np.asarray(lin_r_w[l], np.float32) * scale[:, None]).T)
        bb.append(np.asarray(lin_l_b[l], np.float32) * scale
                  + np.asarray(bn_b[l], np.float32))
    wl = np.stack(wl).astype(nbf)   # [3,128fin,128fout]
    wr = np.stack(wr).astype(nbf)
    bb = np.stack(bb, axis=1).astype(np.float32)  # [128,3]
    return wl, wr, bb


def _build(S, emulate_collective=False):
    TSTR = 8 * K_R + S
    T = 24 * TSTR + 4 * K_R + S
    TREG = NGRP * K_R
    nc = bass.Bass(dynamic_dma_scratch_size=65536)
    idx = nc.declare_dram_parameter("idx", [128, T], i32, isOutput=False)
    ind = nc.declare_dram_parameter("ind", [128, TREG * GW], f8, isOutput=False)
    scol = nc.declare_dram_parameter("scol", [128, NCHUNK * S], f32,
                                      isOutput=False)
    iota = nc.declare_dram_parameter("iota", [128, 512], f32, isOutput=False)
    xg = nc.declare_dram_parameter("xg", [128, T * 128], f8, isOutput=False)
    xT = nc.declare_dram_parameter("xT", [128, PADN], bf16, isOutput=False)
    dvr = nc.declare_dram_parameter("dvr", [1, PADN], bf16, isOutput=False)
    wl = nc.declare_dram_parameter("wl", [3 * 128, 128], bf16, isOutput=False)
    bbr = nc.declare_dram_parameter("bbr", [1, 3 * 128], bf16, isOutput=False)
    wr = nc.declare_dram_parameter("wr", [3 * 128, 128], bf16, isOutput=False)
    bb = nc.declare_dram_parameter("bb", [128, 3], f32, isOutput=False)
    out = nc.declare_dram_parameter("out", [128, 1], f32, isOutput=True)

    # per-slab AllGather overlaps with compute on the remaining chunks;
    # tapered tail slabs so the layer-boundary flush is short.
    SLABS = list(zip(SLAB_FIRST, SLAB_CHUNKS))
    slab_nodes = list(zip(SLAB_NS0, SLAB_NSW))
    shard = [[nc.dram_tensor(f"shard{l}_{si}", [nsw, D], f8)
              for si, (_, nsw) in enumerate(slab_nodes)] for l in range(2)]
    tables = [nc.dram_tensor(f"table{l}", [V, D], f8, addr_space="Shared")
              for l in range(2)]

    Relu = mybir.ActivationFunctionType.Relu

    GB = 32                  # gather batch cap (<=4096 rows/descriptors)

    with tile.TileContext(nc) as tc, ExitStack() as ctx:
        res = ctx.enter_context(tc.tile_pool(name="res", bufs=1))
        gp = ctx.enter_context(tc.tile_pool(name="g", bufs=4))
        aggp = ctx.enter_context(tc.tile_pool(name="agg", bufs=4))
        rowp = ctx.enter_context(tc.tile_pool(name="row", bufs=4))
        psg = ctx.enter_context(tc.tile_pool(name="psg", bufs=3, space="PSUM"))
        psh = ctx.enter_context(tc.tile_pool(name="psh", bufs=2, space="PSUM"))
        pst = ctx.enter_context(tc.tile_pool(name="pst", bufs=2, space="PSUM"))

        # ---- residents (x/ind streamed chunked so chunk 0 starts early)
        idx_sb = res.tile([128, T], i32)
        HT = res.tile([128, PADN], bf16)
        dv_sb = res.tile([128, PADN], bf16)
        ones1 = res.tile([1, 128], bf16)
        wl_sb = res.tile([128, 3 * 128], bf16)
        nc.sync.dma_start(wl_sb[:].rearrange("p (l f) -> p l f", l=3),
                          wl[:].rearrange("(l p) f -> p l f", p=128))
        wr_sb = res.tile([128, 3 * 128], bf16)
        nc.sync.dma_start(wr_sb[:].rearrange("p (l f) -> p l f", l=3),
                          wr[:].rearrange("(l p) f -> p l f", p=128))
        bb_sb = res.tile([128, 3], f32)
        nc.sync.dma_start(bb_sb[:], bb[:])
        bbr_sb = res.tile([1, 3 * 128], bf16)
        nc.sync.dma_start(bbr_sb[:], bbr[:])
        pool_st = res.tile([128, 3 * NCHUNK + 1], f32)
        ind_sb = res.tile([128, TREG * GW], f8)
        isp_sb = res.tile([128, NCHUNK * S * 512], f8)
        iota_sb = res.tile([128, 512], f32)
        scol_sb = res.tile([128, NCHUNK * S], f32)
        nc.sync.dma_start(iota_sb[:], iota[:])
        nc.sync.dma_start(scol_sb[:], scol[:])
        nc.sync.dma_start(dv_sb[0:1, :], dvr[:])
        nc.gpsimd.memset(ones1[:], 1.0)
        # interleave resident loads in consumption order (4-chunk blocks)
        for cb in range(0, NCHUNK, 4):
            r0 = cb * 24 * GW
            r1 = min((cb + 4) * 24, TREG) * GW
            nc.sync.dma_start(ind_sb[:, r0:r1], ind[:, r0:r1])
            for chunk in range(cb, min(cb + 4, NCHUNK)):
                for k in range(S):
                    t = chunk * S + k
                    nc.gpsimd.tensor_scalar(
                        isp_sb[:, t * 512:(t + 1) * 512], iota_sb[:],
                        scol_sb[:, t:t + 1], None, mybir.AluOpType.is_equal)
            c0 = cb * 512
            c1 = min((cb + 4) * 512, PADN)
            nc.sync.dma_start(HT[:, c0:c1], xT[:, c0:c1])
            # replicate deg_inv across partitions: ones[1,128].T @ dvr[1,w]
            for chunk in range(cb, min(cb + 4, NCHUNK)):
                cs = chunk * 512
                w = 512 if chunk < 24 else 256
                psd = psg.tile([128, 512], f32, tag="psg")
                nc.tensor.matmul(psd[:, :w], lhsT=ones1[:],
                                 rhs=dv_sb[0:1, cs:cs + w],
                                 start=True, stop=True)
                nc.vector.tensor_copy(dv_sb[:, cs:cs + w], psd[:, :w])
        nc.sync.dma_start(idx_sb[:], idx[:])

        pair = {}

        def rowpath(l, chunk, agg):
            # node-major h rows computed directly: transposed matmuls
            # (lhsT = agg/HT 128-node blocks, rhs = weights) + K=1 bias
            # matmul + DVE relu. No dependence on the Act engine's HT
            # output, so shard writes never wait on the activation chain.
            cs = chunk * 512
            w = 512 if chunk < 24 else 256
            si = max(i for i, f in enumerate(SLAB_FIRST) if f <= chunk)
            ns0, nsw = slab_nodes[si]
            if chunk % 2 == 0:
                rowt = rowp.tile([128, 1024], f8, tag="row")
                pair["row"] = rowt
            row = pair["row"]
            off = 512 * (chunk % 2)
            ps2 = pst.tile([128, 512], f32, tag="pst")
            for b in range(w // 128):
                c0 = cs + b * 128
                o = ps2[:, b * 128:(b + 1) * 128]
                nc.tensor.matmul(o, lhsT=ones1[:],
                                 rhs=bbr_sb[0:1, l * 128:(l + 1) * 128],
                                 start=True, stop=False)
                nc.tensor.matmul(o, lhsT=agg[:, b * 128:(b + 1) * 128],
                                 rhs=wl_sb[:, l * 128:(l + 1) * 128],
                                 start=False, stop=False)
                nc.tensor.matmul(o, lhsT=HT[:, c0:c0 + 128],
                                 rhs=wr_sb[:, l * 128:(l + 1) * 128],
                                 start=False, stop=True)
            nc.vector.tensor_relu(row[:, off:off + w], ps2[:, :w])
            if chunk % 2 == 1 or chunk == 24:
                base = (chunk - chunk % 2) * 512
                wp = off + w
                nc.sync.dma_start(
                    shard[l][si][base - ns0:base - ns0 + wp, :]
                    .rearrange("(j p) f -> p j f", p=128),
                    row[:, :wp].rearrange("p (j f) -> p j f", f=128))
            if chunk == SLABS[si][0] + SLABS[si][1] - 1:
                if emulate_collective:
                    # equivalent-volume emulation: seed own block then double
                    # (1+1+2+4 blocks = the 8 slab-blocks an AllGather writes)
                    eng = nc.scalar if si % 2 else nc.sync
                    b0 = 8 * ns0
                    eng.dma_start(tables[l][b0:b0 + nsw, :], shard[l][si][:])
                    for dd in (1, 2, 4):
                        eng.dma_start(
                            tables[l][b0 + dd * nsw:b0 + 2 * dd * nsw, :],
                            tables[l][b0:b0 + dd * nsw, :])
                else:
                    nc.gpsimd.collective_compute(
                        "AllGather", mybir.AluOpType.bypass,
                        ins=[shard[l][si][:]],
                        outs=[tables[l][8 * ns0:8 * ns0 + P * nsw, :]],
                        replica_groups=[list(range(P))])

        for l in range(3):
            table_r = tables[l - 1] if l > 0 else None
            for chunk in range(NCHUNK):
                cs = chunk * 512
                w = 512 if chunk < 24 else 256
                ngr = 8 if chunk < 24 else 4
                nreg = ngr * K_R
                t0 = chunk * TSTR
                nt = nreg + S
                rbase = chunk * 24 if chunk < 24 else 576
                gblk = gp.tile([128, TSTR * 128], f8, tag="g")
                if l == 0:
                    nc.scalar.dma_start(gblk[:, :nt * 128],
                                        xg[:, t0 * 128:(t0 + nt) * 128])
                else:
                    for b in range(0, nt, GB):
                        be = min(b + GB, nt)
                        nc.gpsimd.indirect_dma_start(
                            out=gblk[:, b * 128:be * 128],
                            out_offset=None, in_=table_r[:],
                            in_offset=bass.IndirectOffsetOnAxis(
                                ap=idx_sb[:, t0 + b:t0 + be], axis=0))
                ps = psg.tile([128, 512], f32, tag="psg")
                for k in range(S):   # spill tiles first: zero the region
                    sc0 = (chunk * S + k) * 512
                    nc.tensor.matmul(
                        ps[:, :w], lhsT=gblk[:, (nreg + k) * 128:
                                             (nreg + k + 1) * 128],
                        rhs=isp_sb[:, sc0:sc0 + w],
                        start=(k == 0), stop=False, skip_group_check=True)
                for gi in range(ngr):
                    for kk in range(K_R):
                        ti = gi * K_R + kk
                        nc.tensor.matmul(
                            ps[:, gi * GW:(gi + 1) * GW],
                            lhsT=gblk[:, ti * 128:(ti + 1) * 128],
                            rhs=ind_sb[:, (rbase + ti) * GW:
                                       (rbase + ti + 1) * GW],
                            start=False, stop=(kk == K_R - 1),
                            skip_group_check=True)
                agg = aggp.tile([128, 512], bf16, tag="agg")
                nc.vector.tensor_mul(agg[:, :w], ps[:, :w],
                                     dv_sb[:, cs:cs + w])

                ph = psh.tile([128, 512], f32, tag="psh")
                nc.tensor.matmul(ph[:, :w], lhsT=wl_sb[:, l * 128:(l + 1) * 128],
                                 rhs=agg[:, :w], start=True, stop=False)
                nc.tensor.matmul(ph[:, :w], lhsT=wr_sb[:, l * 128:(l + 1) * 128],
                                 rhs=HT[:, cs:cs + w], start=False, stop=True)
                if l < 2:
                    rowpath(l, chunk, agg)
                pcol = pool_st[:, l * NCHUNK + chunk:l * NCHUNK + chunk + 1]
                if chunk < 24:
                    nc.scalar.activation(HT[:, cs:cs + w], ph[:, :w], Relu,
                                         bias=bb_sb[:, l:l + 1], accum_out=pcol)
                else:
                    nc.scalar.activation(HT[:, cs:cs + 212], ph[:, :212], Relu,
                                         bias=bb_sb[:, l:l + 1], accum_out=pcol)
                    nc.scalar.activation(HT[:, cs + 212:cs + 256],
                                         ph[:, 212:256], Relu,
                                         bias=bb_sb[:, l:l + 1])

        nc.vector.reduce_sum(pool_st[:, 3 * NCHUNK:], pool_st[:, :3 * NCHUNK],
                             axis=mybir.AxisListType.X)
        outp = res.tile([128, 1], f32)
        nc.vector.tensor_copy(outp[:], pool_st[:, 3 * NCHUNK:])
        nc.sync.dma_start(out[:], outp[:])
    _split_multi_waits(nc)
    return nc


# ---------------------------------------------------------------------------
def _make_runner(nc, n_cores=P):
    import jax
    from jax.sharding import Mesh, PartitionSpec
    try:
        from jax.experimental.shard_map import shard_map
    except ImportError:
        from jax.shard_map import shard_map
    from concourse import bass2jax
    from concourse.bass2jax import _bass_exec_p, partition_id_tensor

    bass2jax.install_neuronx_cc_hook()
    partition_name = nc.partition_id_tensor.name if nc.partition_id_tensor else None
    in_names, out_names, out_avals, zero_outs = [], [], [], []
    for alloc in nc.m.functions[0].allocations:
        if not isinstance(alloc, mybir.MemoryLocationSet):
            continue
        name = alloc.memorylocations[0].name
        if alloc.kind == "ExternalInput":
            if name != partition_name:
                in_names.append(name)
        elif alloc.kind == "ExternalOutput":
            out_names.append(name)
            shape = tuple(alloc.tensor_shape)
            dtype = mybir.dt.np(alloc.dtype)
            out_avals.append(jax.core.ShapedArray(shape, dtype))
            zero_outs.append(np.zeros(shape, dtype))
    n_params = len(in_names)
    in_names_all = list(in_names) + list(out_names)
    if partition_name is not None:
        in_names_all.append(partition_name)

    def _body(*args):
        operands = list(args)
        if partition_name is not None:
            operands.append(partition_id_tensor())
        return tuple(_bass_exec_p.bind(
            *operands, out_avals=tuple(out_avals), in_names=tuple(in_names_all),
            out_names=tuple(out_names), lowering_input_output_aliases=(),
            sim_require_finite=True, sim_require_nnan=True, nc=nc))

    devices = jax.devices()[:n_cores]
    mesh = Mesh(np.asarray(devices), ("core",))
    nspec = n_params + len(out_names)
    sharded = jax.jit(
        shard_map(_body, mesh=mesh,
                  in_specs=(PartitionSpec("core"),) * nspec,
                  out_specs=(PartitionSpec("core"),) * len(out_names),
                  check_rep=False),
        keep_unused=True)

    def run(in_maps):
        per_core = [[np.asarray(m[name]) for name in in_names] for m in in_maps]
        concat_in = [np.concatenate([per_core[c][i] for c in range(n_cores)], axis=0)
                     for i in range(n_params)]
        concat_zeros = [np.zeros((n_cores * z.shape[0], *z.shape[1:]), z.dtype)
                        for z in zero_outs]
        args = concat_in + concat_zeros
        out_arrs = sharded(*args)
        jax.block_until_ready(out_arrs)
        return [{name: np.asarray(out_arrs[i]).reshape(n_cores,
                                                       *out_avals[i].shape)[c]
                 for i, name in enumerate(out_names)}
                for c in range(n_cores)], (sharded, args)
    return run


_CACHE = {}


def kernel(x, lin_l_w, lin_l_b, lin_r_w, bn_w, bn_b,
           fc1_w, fc1_b, fc2_w, fc2_b, fc3_w, fc3_b, edge_index):
    x = np.asarray(x, np.float32)
    per_core, S, _ = _host_prep(x, edge_index)
    wlw, wrw, bbw = _fold_weights(lin_l_w, lin_l_b, lin_r_w, bn_w, bn_b)
    wl_in = np.ascontiguousarray(wlw.reshape(3 * 128, 128))
    wr_in = np.ascontiguousarray(wrw.reshape(3 * 128, 128))

    if S not in _CACHE:
        nc = _build(S)
        _CACHE[S] = _make_runner(nc)
    run = _CACHE[S]

    bbr_in = np.ascontiguousarray(bbw.T.reshape(1, 3 * 128)).astype(nbf)
    in_maps = [{**per_core[c], "wl": wl_in, "wr": wr_in, "bb": bbw,
                "bbr": bbr_in}
               for c in range(P)]
    res, _ = run(in_maps)

    g = x.sum(axis=0, dtype=np.float64).astype(np.float32)
    for c in range(P):
        g = g + res[c]["out"][:, 0]

    fc1_w = np.asarray(fc1_w, np.float32); fc1_b = np.asarray(fc1_b, np.float32)
    fc2_w = np.asarray(fc2_w, np.float32); fc2_b = np.asarray(fc2_b, np.float32)
    fc3_w = np.asarray(fc3_w, np.float32); fc3_b = np.asarray(fc3_b, np.float32)
    h = np.maximum(g @ fc1_w.T + fc1_b, 0.0)
    h = np.maximum(h @ fc2_w.T + fc2_b, 0.0)
    o = h @ fc3_w.T + fc3_b
    return o[None, :].astype(np.float32)



# revision 61
# speedup vs baseline: 1.0016x; 1.0016x over previous
"""GraphSAGE GNN (N=100k, E=600k, D=128, L=4) on 8 Trainium2 NeuronCores.

Strategy (memory-regime):
- Layer 3 of the reference is dead code (its output never reaches node_rep):
  only 3 SAGE layers are computed.
- Nodes sharded contiguously: core c owns dsts [c*12500,(c+1)*12500), padded
  to 12544 rows. Edges sorted by dst on host (graph partitioning): each
  64-dst group gets 3 regular 128-edge tiles; overflow edges go to S shared
  per-chunk spill tiles (one-hot spans the whole 512-dst chunk) -> ~92%
  slot fill with an identical SPMD instruction stream on all cores.
- Mean-aggregation = PE matmuls: psum[feat,dst] += G_tile.T @ IND_tile where
  G = gathered source rows (fp8e4m3, 128B/row) and IND holds exact 0/1
  one-hot columns in fp8; deg_inv applied afterwards as an elementwise
  multiply against a bf16 tile broadcast on-device from a [1, PADN] row
  via a K=1 PE outer product (exact bf16 mean, fp8 only on transported h).
  Spill-tile one-hots are built on-device (gpsimd iota==col compare) from
  a tiny column-index array instead of streaming 3.2MB of mostly zeros.
- Indirect gathers are batched one chunk (26 tiles, 3328 rows) per SWDGE
  instruction (64KB descriptor scratch) to amortize the ~1us fixed
  desc-gen cost on Pool; transfers run on the 16 DMA engines.
- Layer 0's gather is precomputed on host (x[src] in fp8) and streamed in
  partition-major layout [128, T*128] so DMA runs are 4KB (full bandwidth).
- h tables replicated via per-slab AllGather (fp8, slab-major table layout
  so each slab's output is contiguous), overlapped with remaining chunks.
- Node-major shard rows computed directly by transposed matmuls (lhsT =
  agg/HT 128-node blocks, rhs = weights, bias via a K=1 ones-matmul into
  PSUM, bias-less DVE relu) — no PE transposes, and shard writes never
  wait on the Act engine's chain.
- BN (eval) folded into weights/bias on host. Activations kept transposed
  [feat, node]; per-layer JK pooling via the ACT accum_out side-output.
- Final global pool partials [128] per core summed on host + tiny MLP head
  on host (0.1% of FLOPs).
"""
import numpy as np
import ml_dtypes
from contextlib import ExitStack

import concourse.bass as bass
import concourse.tile as tile
import concourse.tile as tile_mod
from concourse import mybir
from concourse.vector_clock import ScopedClock

# ---------------------------------------------------------------------------
# Walrus workaround: this compiler rejects >1 sem wait on CTRL_NO instructions
# (Drain/NoOp). Tile's final drain carries one wait per active proc — split
# them one-per-nop and emit a waitless drain.
def _drain_and_barrier(self, tick_clock, wait_clock):
    probe = self.nc.sync.nop(hint="pre_drain_waits", nofuse=True)
    wait_clock.add_sem_waits(probe.ins, ScopedClock({None: tick_clock.global_clock}))
    si = probe.ins.sync_info
    waits = list(si.on_wait) if si is not None else []
    if len(waits) > 1:
        probe.ins.sync_info = mybir.SyncInfo(on_wait=waits[:1], on_update=[])
        for w in waits[1:]:
            extra = self.nc.sync.nop(hint="pre_drain_waits_x", nofuse=True)
            extra.ins.sync_info = mybir.SyncInfo(on_wait=[w], on_update=[])
    self.nc.sync.drain()
    self.nc.all_engine_barrier()
    assert self.sems is not None
    popped = self.nc._tile_sem_poison_stack.pop()
    assert popped is self._sem_poison
    self.nc.clear_and_free_semaphores(list(self.sems.allocated().values()))
    self.nc.all_engine_barrier()


tile_mod.TileContext._drain_and_barrier = _drain_and_barrier


def _split_multi_waits(nc):
    """This walrus build allows at most ONE sem wait per instruction. Tile
    emits several on some. Split: carrier nops (same engine, program order
    preserved) take all but the last wait."""
    targets = []
    for f in nc.m.functions:
        for blk in f.blocks:
            for inst in blk.instructions:
                si = inst.sync_info
                if si is not None and len(si.on_wait) > 1:
                    targets.append((blk, inst))
    if not targets:
        return
    carriers = {}  # inst name -> list of carrier insts
    created = []
    for blk, inst in targets:
        waits = list(inst.sync_info.on_wait)
        cs = []
        for w in waits[:-1]:
            c = nc.engines[inst.engine].nop(hint="wsplit", nofuse=True)
            c.ins.sync_info = mybir.SyncInfo(on_wait=[w], on_update=[])
            cs.append(c.ins)
            created.append(c.ins.name)
        inst.sync_info = mybir.SyncInfo(
            on_wait=[waits[-1]], on_update=list(inst.sync_info.on_update))
        carriers[inst.name] = cs
    created = set(created)
    for f in nc.m.functions:
        for blk in f.blocks:
            insts = list(blk.instructions)
            new = []
            changed = False
            for inst in insts:
                if inst.name in created:
                    changed = True
                    continue  # remove from tail position
                if inst.name in carriers:
                    new.extend(carriers[inst.name])
                    changed = True
                new.append(inst)
            if changed:
                blk.instructions = new

# ---------------------------------------------------------------------------
N, E, D, L = 100000, 600000, 128, 4
P = 8
NPC = N // P            # 12500 nodes per core
PADN = 12544            # padded to 98*128
V = PADN * P            # padded global table rows
GW = 64                 # dst-group width
NGRP = PADN // GW       # 196 groups per core
BN_EPS = 1e-5
NCHUNK = 25             # 24x512 + 1x256 node chunks
K_R = 3                 # regular 128-edge tiles per 64-dst group
SLAB_CHUNKS = [8, 8, 4, 2, 2, 1]  # chunks per AllGather slab (tapered tail)
SLAB_FIRST = [0, 8, 16, 20, 22, 24]
SLAB_NS0 = [0, 4096, 8192, 10240, 11264, 12288]
SLAB_NSW = [4096, 4096, 2048, 1024, 1024, 256]
bf16 = mybir.dt.bfloat16
f8 = mybir.dt.float8e4
f32 = mybir.dt.float32
i32 = mybir.dt.int32
nbf = ml_dtypes.bfloat16
nf8 = ml_dtypes.float8_e4m3


def _host_prep(x, edge_index):
    src = np.asarray(edge_index[0], dtype=np.int64)
    dst = np.asarray(edge_index[1], dtype=np.int64)
    deg = np.bincount(dst, minlength=N).astype(np.float64)
    deg_inv = np.where(deg > 0, 1.0 / np.maximum(deg, 1.0), 0.0).astype(np.float32)

    order = np.argsort(dst, kind="stable")
    ds = dst[order]
    ss = src[order]
    core_bounds = np.searchsorted(ds, np.arange(P + 1) * NPC)

    # per (core, group) edge counts. Each 64-dst group gets K_R=3 regular
    # tiles (384 slots); overflow edges spill into S shared per-chunk tiles
    # whose one-hot IND spans the whole 512-dst chunk. S is data-derived.
    percore = []
    max_spill = 0
    for c in range(P):
        lo, hi = core_bounds[c], core_bounds[c + 1]
        l = (ds[lo:hi] - c * NPC).astype(np.int64)
        s = ss[lo:hi]
        gb = np.searchsorted(l, np.arange(NGRP + 1) * GW)
        cnt = np.diff(gb)
        ov = np.maximum(cnt - K_R * 128, 0)
        for ch in range(NCHUNK):
            g0, g1 = ch * 8, min(ch * 8 + 8, NGRP)
            max_spill = max(max_spill, int(ov[g0:g1].sum()))
        percore.append((l, s, gb))
    S = (max_spill + 127) // 128
    TSTR = 8 * K_R + S           # tile stride per chunk
    T = 24 * TSTR + 4 * K_R + S  # 24 full chunks + final half chunk
    TREG = NGRP * K_R            # regular tiles total (588)

    # table rows are slab-major (slab, core, node-in-slab) so each slab's
    # AllGather lands in one contiguous output range
    _b = np.asarray(SLAB_NS0 + [PADN])
    _w = np.asarray(SLAB_NSW)
    def _srow(glob):
        sc = glob // NPC
        local = glob % NPC
        si = np.searchsorted(_b, local, "right") - 1
        return 8 * _b[si] + sc * _w[si] + (local - _b[si])

    xf = np.asarray(x, dtype=np.float32)

    ins = []
    for c in range(P):
        lo, hi = core_bounds[c], core_bounds[c + 1]
        l, s, gb = percore[c]
        sr = _srow(ss[lo:hi])

        idx_t = np.zeros((T, 128), np.int32)            # tile, edge-in-tile
        ind_t = np.zeros((TREG, 128, GW), np.float32)   # regular tiles
        spcol = np.full((NCHUNK * S, 128), 1e9, np.float32)  # spill dst cols
        xsrc_t = np.zeros((T * 128,), np.int64) - 1     # original src per slot
        for ch in range(NCHUNK):
            ngr = 8 if ch < 24 else 4
            nreg = ngr * K_R
            t0 = ch * TSTR
            spfill = 0
            for gi in range(ngr):
                g = ch * 8 + gi
                e0, e1 = gb[g], gb[g + 1]
                n = e1 - e0
                if n == 0:
                    continue
                rows = sr[e0:e1]
                gcols = (l[e0:e1] - g * GW).astype(np.int64)
                ccols = (l[e0:e1] - ch * 512).astype(np.int64)
                nr = min(n, K_R * 128)
                for kk in range((nr + 127) // 128):
                    a, b = kk * 128, min((kk + 1) * 128, nr)
                    pp = np.arange(a, b) - a
                    t = t0 + gi * K_R + kk
                    rt = (ch * 24 if ch < 24 else 576) + gi * K_R + kk
                    idx_t[t, pp] = rows[a:b]
                    ind_t[rt, pp, gcols[a:b]] = 1.0
                    xsrc_t[t * 128 + pp] = s[e0 + a:e0 + b]
                for e in range(nr, n):      # spill
                    k, slot = spfill // 128, spfill % 128
                    t = t0 + nreg + k
                    idx_t[t, slot] = rows[e]
                    spcol[ch * S + k, slot] = ccols[e]
                    xsrc_t[t * 128 + slot] = s[e0 + e]
                    spfill += 1
            assert spfill <= S * 128

        idx_in = np.ascontiguousarray(idx_t.T)                    # [128, T]
        ind_in = np.ascontiguousarray(
            ind_t.transpose(1, 0, 2).reshape(128, TREG * GW)).astype(nf8)
        scol_in = np.ascontiguousarray(spcol.T)                   # [128, 25*S]
        dvrow = np.zeros((PADN,), np.float32)
        dvrow[:NPC] = deg_inv[c * NPC:(c + 1) * NPC]
        dv_in = dvrow[None, :].astype(nbf)
        xg0 = np.zeros((T * 128, D), nf8)
        valid = xsrc_t >= 0
        xg0[valid] = xf[xsrc_t[valid]].astype(nf8)
        # partition-major: xg[p, t*128+f] = x[src(tile t, slot p)][f]
        xg = np.ascontiguousarray(
            xg0.reshape(T, 128, D).transpose(1, 0, 2).reshape(128, T * D))

        xT = np.zeros((128, PADN), np.float32)
        xT[:, :NPC] = xf[c * NPC:(c + 1) * NPC].T
        ins.append({
            "idx": idx_in,
            "ind": ind_in,
            "scol": scol_in,
            "iota": np.ascontiguousarray(np.broadcast_to(
                np.arange(512, dtype=np.float32)[None, :], (128, 512))),
            "xg": xg,
            "xT": np.ascontiguousarray(xT.astype(nbf)),
            "dvr": np.ascontiguousarray(dv_in[:1]),
        })
    return ins, S, deg_inv


def _fold_weights(lin_l_w, lin_l_b, lin_r_w, bn_w, bn_b):
    inv_std = 1.0 / np.sqrt(1.0 + BN_EPS)
    wl, wr, bb = [], [], []
    for l in range(3):
        scale = (np.asarray(bn_w[l], np.float32) * inv_std)
        wl.append((np.asarray(lin_l_w[l], np.float32) * scale[:, None]).T)
        wr.append((np.asarray(lin_r_w[l], np.float32) * scale[:, None]).T)
        bb.append(np.asarray(lin_l_b[l], np.float32) * scale
                  + np.asarray(bn_b[l], np.float32))
    wl = np.stack(wl).astype(nbf)   # [3,128fin,128fout]
    wr = np.stack(wr).astype(nbf)
    bb = np.stack(bb, axis=1).astype(np.float32)  # [128,3]
    return wl, wr, bb


def _build(S, emulate_collective=False):
    TSTR = 8 * K_R + S
    T = 24 * TSTR + 4 * K_R + S
    TREG = NGRP * K_R
    nc = bass.Bass(dynamic_dma_scratch_size=65536)
    idx = nc.declare_dram_parameter("idx", [128, T], i32, isOutput=False)
    ind = nc.declare_dram_parameter("ind", [128, TREG * GW], f8, isOutput=False)
    scol = nc.declare_dram_parameter("scol", [128, NCHUNK * S], f32,
                                      isOutput=False)
    iota = nc.declare_dram_parameter("iota", [128, 512], f32, isOutput=False)
    xg = nc.declare_dram_parameter("xg", [128, T * 128], f8, isOutput=False)
    xT = nc.declare_dram_parameter("xT", [128, PADN], bf16, isOutput=False)
    dvr = nc.declare_dram_parameter("dvr", [1, PADN], bf16, isOutput=False)
    wl = nc.declare_dram_parameter("wl", [3 * 128, 128], bf16, isOutput=False)
    bbr = nc.declare_dram_parameter("bbr", [1, 3 * 128], bf16, isOutput=False)
    wr = nc.declare_dram_parameter("wr", [3 * 128, 128], bf16, isOutput=False)
    bb = nc.declare_dram_parameter("bb", [128, 3], f32, isOutput=False)
    out = nc.declare_dram_parameter("out", [128, 1], f32, isOutput=True)

    # per-slab AllGather overlaps with compute on the remaining chunks;
    # tapered tail slabs so the layer-boundary flush is short.
    SLABS = list(zip(SLAB_FIRST, SLAB_CHUNKS))
    slab_nodes = list(zip(SLAB_NS0, SLAB_NSW))
    shard = [[nc.dram_tensor(f"shard{l}_{si}", [nsw, D], f8)
              for si, (_, nsw) in enumerate(slab_nodes)] for l in range(2)]
    tables = [nc.dram_tensor(f"table{l}", [V, D], f8, addr_space="Shared")
              for l in range(2)]

    Relu = mybir.ActivationFunctionType.Relu

    GB = 32                  # gather batch cap (<=4096 rows/descriptors)

    with tile.TileContext(nc) as tc, ExitStack() as ctx:
        res = ctx.enter_context(tc.tile_pool(name="res", bufs=1))
        gp = ctx.enter_context(tc.tile_pool(name="g", bufs=5))
        aggp = ctx.enter_context(tc.tile_pool(name="agg", bufs=5))
        rowp = ctx.enter_context(tc.tile_pool(name="row", bufs=4))
        psg = ctx.enter_context(tc.tile_pool(name="psg", bufs=3, space="PSUM"))
        psh = ctx.enter_context(tc.tile_pool(name="psh", bufs=2, space="PSUM"))
        pst = ctx.enter_context(tc.tile_pool(name="pst", bufs=2, space="PSUM"))

        # ---- residents (x/ind streamed chunked so chunk 0 starts early)
        idx_sb = res.tile([128, T], i32)
        HT = res.tile([128, PADN], bf16)
        dv_sb = res.tile([128, PADN], bf16)
        ones1 = res.tile([1, 128], bf16)
        wl_sb = res.tile([128, 3 * 128], bf16)
        nc.sync.dma_start(wl_sb[:].rearrange("p (l f) -> p l f", l=3),
                          wl[:].rearrange("(l p) f -> p l f", p=128))
        wr_sb = res.tile([128, 3 * 128], bf16)
        nc.sync.dma_start(wr_sb[:].rearrange("p (l f) -> p l f", l=3),
                          wr[:].rearrange("(l p) f -> p l f", p=128))
        bb_sb = res.tile([128, 3], f32)
        nc.sync.dma_start(bb_sb[:], bb[:])
        bbr_sb = res.tile([1, 3 * 128], bf16)
        nc.sync.dma_start(bbr_sb[:], bbr[:])
        pool_st = res.tile([128, 3 * NCHUNK + 1], f32)
        ind_sb = res.tile([128, TREG * GW], f8)
        isp_sb = res.tile([128, NCHUNK * S * 512], f8)
        iota_sb = res.tile([128, 512], f32)
        scol_sb = res.tile([128, NCHUNK * S], f32)
        nc.sync.dma_start(iota_sb[:], iota[:])
        nc.sync.dma_start(scol_sb[:], scol[:])
        nc.sync.dma_start(dv_sb[0:1, :], dvr[:])
        nc.gpsimd.memset(ones1[:], 1.0)
        # interleave resident loads in consumption order (4-chunk blocks)
        for cb in range(0, NCHUNK, 4):
            r0 = cb * 24 * GW
            r1 = min((cb + 4) * 24, TREG) * GW
            nc.sync.dma_start(ind_sb[:, r0:r1], ind[:, r0:r1])
            for chunk in range(cb, min(cb + 4, NCHUNK)):
                for k in range(S):
                    t = chunk * S + k
                    nc.gpsimd.tensor_scalar(
                        isp_sb[:, t * 512:(t + 1) * 512], iota_sb[:],
                        scol_sb[:, t:t + 1], None, mybir.AluOpType.is_equal)
            c0 = cb * 512
            c1 = min((cb + 4) * 512, PADN)
            nc.sync.dma_start(HT[:, c0:c1], xT[:, c0:c1])
            # replicate deg_inv across partitions: ones[1,128].T @ dvr[1,w]
            for chunk in range(cb, min(cb + 4, NCHUNK)):
                cs = chunk * 512
                w = 512 if chunk < 24 else 256
                psd = psg.tile([128, 512], f32, tag="psg")
                nc.tensor.matmul(psd[:, :w], lhsT=ones1[:],
                                 rhs=dv_sb[0:1, cs:cs + w],
                                 start=True, stop=True)
                nc.vector.tensor_copy(dv_sb[:, cs:cs + w], psd[:, :w])
        nc.sync.dma_start(idx_sb[:], idx[:])

        pair = {}

        def rowpath(l, chunk, agg):
            # node-major h rows computed directly: transposed matmuls
            # (lhsT = agg/HT 128-node blocks, rhs = weights) + K=1 bias
            # matmul + DVE relu. No dependence on the Act engine's HT
            # output, so shard writes never wait on the activation chain.
            cs = chunk * 512
            w = 512 if chunk < 24 else 256
            si = max(i for i, f in enumerate(SLAB_FIRST) if f <= chunk)
            ns0, nsw = slab_nodes[si]
            if chunk % 2 == 0:
                rowt = rowp.tile([128, 1024], f8, tag="row")
                pair["row"] = rowt
            row = pair["row"]
            off = 512 * (chunk % 2)
            ps2 = pst.tile([128, 512], f32, tag="pst")
            for b in range(w // 128):
                c0 = cs + b * 128
                o = ps2[:, b * 128:(b + 1) * 128]
                nc.tensor.matmul(o, lhsT=ones1[:],
                                 rhs=bbr_sb[0:1, l * 128:(l + 1) * 128],
                                 start=True, stop=False)
                nc.tensor.matmul(o, lhsT=agg[:, b * 128:(b + 1) * 128],
                                 rhs=wl_sb[:, l * 128:(l + 1) * 128],
                                 start=False, stop=False)
                nc.tensor.matmul(o, lhsT=HT[:, c0:c0 + 128],
                                 rhs=wr_sb[:, l * 128:(l + 1) * 128],
                                 start=False, stop=True)
            nc.vector.tensor_relu(row[:, off:off + w], ps2[:, :w])
            if chunk % 2 == 1 or chunk == 24:
                base = (chunk - chunk % 2) * 512
                wp = off + w
                nc.sync.dma_start(
                    shard[l][si][base - ns0:base - ns0 + wp, :]
                    .rearrange("(j p) f -> p j f", p=128),
                    row[:, :wp].rearrange("p (j f) -> p j f", f=128))
            if chunk == SLABS[si][0] + SLABS[si][1] - 1:
                if emulate_collective:
                    # equivalent-volume emulation: seed own block then double
                    # (1+1+2+4 blocks = the 8 slab-blocks an AllGather writes)
                    eng = nc.scalar if si % 2 else nc.sync
                    b0 = 8 * ns0
                    eng.dma_start(tables[l][b0:b0 + nsw, :], shard[l][si][:])
                    for dd in (1, 2, 4):
                        eng.dma_start(
                            tables[l][b0 + dd * nsw:b0 + 2 * dd * nsw, :],
                            tables[l][b0:b0 + dd * nsw, :])
                else:
                    nc.gpsimd.collective_compute(
                        "AllGather", mybir.AluOpType.bypass,
                        ins=[shard[l][si][:]],
                        outs=[tables[l][8 * ns0:8 * ns0 + P * nsw, :]],
                        replica_groups=[list(range(P))])

        for l in range(3):
            table_r = tables[l - 1] if l > 0 else None
            for chunk in range(NCHUNK):
                cs = chunk * 512
                w = 512 if chunk < 24 else 256
                ngr = 8 if chunk < 24 else 4
                nreg = ngr * K_R
                t0 = chunk * TSTR
                nt = nreg + S
                rbase = chunk * 24 if chunk < 24 else 576
                gblk = gp.tile([128, TSTR * 128], f8, tag="g")
                if l == 0:
                    nc.scalar.dma_start(gblk[:, :nt * 128],
                                        xg[:, t0 * 128:(t0 + nt) * 128])
                else:
                    for b in range(0, nt, GB):
                        be = min(b + GB, nt)
                        nc.gpsimd.indirect_dma_start(
                            out=gblk[:, b * 128:be * 128],
                            out_offset=None, in_=table_r[:],
                            in_offset=bass.IndirectOffsetOnAxis(
                                ap=idx_sb[:, t0 + b:t0 + be], axis=0))
                ps = psg.tile([128, 512], f32, tag="psg")
                for k in range(S):   # spill tiles first: zero the region
                    sc0 = (chunk * S + k) * 512
                    nc.tensor.matmul(
                        ps[:, :w], lhsT=gblk[:, (nreg + k) * 128:
                                             (nreg + k + 1) * 128],
                        rhs=isp_sb[:, sc0:sc0 + w],
                        start=(k == 0), stop=False, skip_group_check=True)
                for gi in range(ngr):
                    for kk in range(K_R):
                        ti = gi * K_R + kk
                        nc.tensor.matmul(
                            ps[:, gi * GW:(gi + 1) * GW],
                            lhsT=gblk[:, ti * 128:(ti + 1) * 128],
                            rhs=ind_sb[:, (rbase + ti) * GW:
                                       (rbase + ti + 1) * GW],
                            start=False, stop=(kk == K_R - 1),
                            skip_group_check=True)
                agg = aggp.tile([128, 512], bf16, tag="agg")
                nc.vector.tensor_mul(agg[:, :w], ps[:, :w],
                                     dv_sb[:, cs:cs + w])

                ph = psh.tile([128, 512], f32, tag="psh")
                nc.tensor.matmul(ph[:, :w], lhsT=wl_sb[:, l * 128:(l + 1) * 128],
                                 rhs=agg[:, :w], start=True, stop=False)
                nc.tensor.matmul(ph[:, :w], lhsT=wr_sb[:, l * 128:(l + 1) * 128],
                                 rhs=HT[:, cs:cs + w], start=False, stop=True)
                if l < 2:
                    rowpath(l, chunk, agg)
                pcol = pool_st[:, l * NCHUNK + chunk:l * NCHUNK + chunk + 1]
                if chunk < 24:
                    nc.scalar.activation(HT[:, cs:cs + w], ph[:, :w], Relu,
                                         bias=bb_sb[:, l:l + 1], accum_out=pcol)
                else:
                    nc.scalar.activation(HT[:, cs:cs + 212], ph[:, :212], Relu,
                                         bias=bb_sb[:, l:l + 1], accum_out=pcol)
                    nc.scalar.activation(HT[:, cs + 212:cs + 256],
                                         ph[:, 212:256], Relu,
                                         bias=bb_sb[:, l:l + 1])

        nc.vector.reduce_sum(pool_st[:, 3 * NCHUNK:], pool_st[:, :3 * NCHUNK],
                             axis=mybir.AxisListType.X)
        outp = res.tile([128, 1], f32)
        nc.vector.tensor_copy(outp[:], pool_st[:, 3 * NCHUNK:])
        nc.sync.dma_start(out[:], outp[:])
    _split_multi_waits(nc)
    return nc


# ---------------------------------------------------------------------------
def _make_runner(nc, n_cores=P):
    import jax
    from jax.sharding import Mesh, PartitionSpec
    try:
        from jax.experimental.shard_map import shard_map
    except ImportError:
        from jax.shard_map import shard_map
    from concourse import bass2jax
    from concourse.bass2jax import _bass_exec_p, partition_id_tensor

    bass2jax.install_neuronx_cc_hook()
    partition_name = nc.partition_id_tensor.name if nc.partition_id_tensor else None
    in_names, out_names, out_avals, zero_outs = [], [], [], []
    for alloc in nc.m.functions[0].allocations:
        if not isinstance(alloc, mybir.MemoryLocationSet):
            continue
        name = alloc.memorylocations[0].name
        if alloc.kind == "ExternalInput":
            if name != partition_name:
                in_names.append(name)
        elif alloc.kind == "ExternalOutput":
            out_names.append(name)
            shape = tuple(alloc.tensor_shape)
            dtype = mybir.dt.np(alloc.dtype)
            out_avals.append(jax.core.ShapedArray(shape, dtype))
            zero_outs.append(np.zeros(shape, dtype))
    n_params = len(in_names)
    in_names_all = list(in_names) + list(out_names)
    if partition_name is not None:
        in_names_all.append(partition_name)

    def _body(*args):
        operands = list(args)
        if partition_name is not None:
            operands.append(partition_id_tensor())
        return tuple(_bass_exec_p.bind(
            *operands, out_avals=tuple(out_avals), in_names=tuple(in_names_all),
            out_names=tuple(out_names), lowering_input_output_aliases=(),
            sim_require_finite=True, sim_require_nnan=True, nc=nc))

    devices = jax.devices()[:n_cores]
    mesh = Mesh(np.asarray(devices), ("core",))
    nspec = n_params + len(out_names)
    sharded = jax.jit(
        shard_map(_body, mesh=mesh,
                  in_specs=(PartitionSpec("core"),) * nspec,
                  out_specs=(PartitionSpec("core"),) * len(out_names),
                  check_rep=False),
        keep_unused=True)

    def run(in_maps):
        per_core = [[np.asarray(m[name]) for name in in_names] for m in in_maps]
        concat_in = [np.concatenate([per_core[c][i] for c in range(n_cores)], axis=0)
                     for i in range(n_params)]
        concat_zeros = [np.zeros((n_cores * z.shape[0], *z.shape[1:]), z.dtype)
                        for z in zero_outs]
        args = concat_in + concat_zeros
        out_arrs = sharded(*args)
        jax.block_until_ready(out_arrs)
        return [{name: np.asarray(out_arrs[i]).reshape(n_cores,
                                                       *out_avals[i].shape)[c]
                 for i, name in enumerate(out_names)}
                for c in range(n_cores)], (sharded, args)
    return run


_CACHE = {}


def kernel(x, lin_l_w, lin_l_b, lin_r_w, bn_w, bn_b,
           fc1_w, fc1_b, fc2_w, fc2_b, fc3_w, fc3_b, edge_index):
    x = np.asarray(x, np.float32)
    per_core, S, _ = _host_prep(x, edge_index)
    wlw, wrw, bbw = _fold_weights(lin_l_w, lin_l_b, lin_r_w, bn_w, bn_b)
    wl_in = np.ascontiguousarray(wlw.reshape(3 * 128, 128))
    wr_in = np.ascontiguousarray(wrw.reshape(3 * 128, 128))

    if S not in _CACHE:
        nc = _build(S)
        _CACHE[S] = _make_runner(nc)
    run = _CACHE[S]

    bbr_in = np.ascontiguousarray(bbw.T.reshape(1, 3 * 128)).astype(nbf)
    in_maps = [{**per_core[c], "wl": wl_in, "wr": wr_in, "bb": bbw,
                "bbr": bbr_in}
               for c in range(P)]
    res, _ = run(in_maps)

    g = x.sum(axis=0, dtype=np.float64).astype(np.float32)
    for c in range(P):
        g = g + res[c]["out"][:, 0]

    fc1_w = np.asarray(fc1_w, np.float32); fc1_b = np.asarray(fc1_b, np.float32)
    fc2_w = np.asarray(fc2_w, np.float32); fc2_b = np.asarray(fc2_b, np.float32)
    fc3_w = np.asarray(fc3_w, np.float32); fc3_b = np.asarray(fc3_b, np.float32)
    h = np.maximum(g @ fc1_w.T + fc1_b, 0.0)
    h = np.maximum(h @ fc2_w.T + fc2_b, 0.0)
    o = h @ fc3_w.T + fc3_b
    return o[None, :].astype(np.float32)

